# revision 2
# baseline (speedup 1.0000x reference)
"""Trainium2 Bass kernel v2: multi-head causal attention with RoPE.

Model (per reference):
  B=2, S=2048, D=4096, H=32 heads, HD=128.
  out = softmax(rope(x@wq) @ rope(x@wk)^T / sqrt(HD) + mask) @ (x@wv) @ wo

Sharding: tensor-parallel over heads. Core c owns heads 4c..4c+3; each core
produces a full-shape partial output, host sums the 8 partials.

v2 design (vs v1 baseline):
  - fp16 storage for x tiles, wq/wk/wv, post-rope Q/K (SBUF-resident,
    no DRAM spill), hoT and wo. fp32 PSUM accumulation everywhere.
    V and probs (ex) stay fp32/f32r.
  - Contiguous chunk-major host layouts for x so every load is one large
    DMA (the cost of a DMA is dominated by per-transfer overheads on the
    shared HWDGE issue path).
  - PSUM ping-pong (bufs=2+) in all phases so PE never waits on drains.
  - Causal mask applied as a multiply on exp(scores) (exp(s+m) =
    exp(s)*exp(m); for the causal mask the factor is exactly 0/1), which
    keeps the mask off the PE->ACT critical path.
  - Attention inner loop software-pipelined (scores run 2 deep ahead of
    the exp-dependent PV/ones matmuls), diagonal blocks processed first.
  - Output projection interleaved with the second batch's attention so
    the PE stays busy while ACT computes exps.
"""

import math
import sys

if "/opt/trn_rl_repo" not in sys.path:
    sys.path.insert(0, "/opt/trn_rl_repo")

import numpy as np

B, S, D, H = 2, 2048, 4096, 32
HD = D // H          # 128
HLOC = 4             # heads per core
NC = 8               # cores
TOK = B * S          # 4096
DKT = D // 128       # 32 contraction tiles
KT = S // 128        # 16 k-tiles per sequence
QC = S // 512        # 4 q-chunks of 512 per sequence
VCH = TOK // 512     # 8 V-pass chunks of 512 tokens
NSC = TOK // 128     # 32 QK-pass sub-chunks of 128 tokens
ISQRT = 1.0 / math.sqrt(HD)

_CACHE = {}


def _build(causal: bool):
    import concourse.bacc as bacc
    import concourse.tile as tile
    from concourse import mybir

    F32 = mybir.dt.float32
    F32R = mybir.dt.float32r
    F16 = mybir.dt.float16
    EXP = mybir.ActivationFunctionType.Exp
    COPY = mybir.ActivationFunctionType.Copy

    nc = bacc.Bacc("TRN2", target_bir_lowering=False, debug=False,
                   num_devices=NC)

    # ---- DRAM inputs ----
    # x tiles partition-major per chunk: one contiguous 8-32KB run per
    # partition per load -> minimal DMA descriptor cost
    xv_d = nc.dram_tensor("xv", [VCH, 128, DKT, 512], F16, kind="ExternalInput")
    xqk_d = nc.dram_tensor("xqk", [NSC, 128, DKT, 128], F16, kind="ExternalInput")
    wq_d = nc.dram_tensor("wq", [128, DKT, 512], F16, kind="ExternalInput")
    wk_d = nc.dram_tensor("wk", [128, DKT, 512], F16, kind="ExternalInput")
    wv_d = nc.dram_tensor("wv", [128, DKT, 512], F16, kind="ExternalInput")
    wo_d = nc.dram_tensor("wo", [128, HLOC, D], F32R, kind="ExternalInput")
    cs_d = nc.dram_tensor("cs", [128, S], F32, kind="ExternalInput")
    ss_d = nc.dram_tensor("ss", [128, S], F32, kind="ExternalInput")
    # exp(mask/sqrt(HD)) factors for the diagonal band (causal) or the
    # full mask (general): multiplied into exp(scores).
    if causal:
        mk_d = nc.dram_tensor("maskd", [128, 4, 512], F32R, kind="ExternalInput")
    else:
        mk_d = nc.dram_tensor("maskf", [KT, 128, S], F32R, kind="ExternalInput")
    out_d = nc.dram_tensor("out", [TOK, D], F32, kind="ExternalOutput")

    import os
    dbg = bool(os.environ.get("K2_DBG"))
    # V spill scratch (f32): [kt, tok%128, dcol]
    vkind = "ExternalOutput" if dbg else "Internal"
    vdr = {b: nc.dram_tensor(f"vdr{b}", [KT, 128, 512], F32R, kind=vkind)
           for b in range(B)}
    # normalized attention output, spilled f32 (no SBUF room at f32)
    hdr = {b: nc.dram_tensor(f"hdbg{b}", [128, HLOC, S], F32R, kind=vkind)
           for b in range(B)}
    qdbg = {}
    if dbg:
        for b in range(B):
            for qk in ("q", "k"):
                qdbg[(qk, b)] = nc.dram_tensor(
                    f"qdbg_{qk}{b}", [128, HLOC, S], F16, kind="ExternalOutput")

    with tile.TileContext(nc) as tc:
        # RIGHT stack: long-lived (consts -> hoT -> qraw b1 -> qraw b0);
        # hoTp/qraw pools are allocated at QK-pass start (a pool reserves
        # its space for its whole lifetime, so allocate late).
        consts = tc.alloc_tile_pool(name="consts", bufs=1, side="right")

        ones_sb = consts.tile([128, 1], F32R)
        nc.vector.memset(ones_sb.bitcast(F32), 1.0)

        # ================= V-pass =================
        # LEFT stack: wqkp (outlives V-pass) under the V-pass pools
        wqkp = tc.alloc_tile_pool(name="wqkp", bufs=1, side="left")
        wvp = tc.alloc_tile_pool(name="wvp", bufs=1, side="left")
        xvp = tc.alloc_tile_pool(name="xvp", bufs=3, side="left")
        vcp = tc.alloc_tile_pool(name="vcp", bufs=2, side="left")
        psV = tc.alloc_tile_pool(name="psV", bufs=2, space="PSUM")

        wv_sb = wvp.tile([128, DKT, 512], F16, tag="wv")
        # wv streamed in pieces, smallest first, so chunk 0 starts ASAP
        _pos = 0
        for w in (2, 2, 4, 4, 4, 4, 4, 4, 4):
            nc.scalar.dma_start(out=wv_sb[:, _pos:_pos + w, :],
                                in_=wv_d.ap()[:, _pos:_pos + w, :])
            _pos += w

        wq_sb = wqkp.tile([128, DKT, 512], F16, tag="wq")
        wk_sb = wqkp.tile([128, DKT, 512], F16, tag="wk")
        xqk0_sb = wqkp.tile([128, DKT, 128], F16, tag="xqk0")

        xv_tiles = {}

        def load_xv(ch, half, split=1):
            t = xvp.tile([128, 16, 512], F16, name=f"xv{ch}_{half}", tag="xv")
            q = 16 // split
            for i in range(split):
                nc.sync.dma_start(
                    out=t[:, i * q:(i + 1) * q, :],
                    in_=xv_d.ap()[ch, :, half * 16 + i * q:
                                  half * 16 + (i + 1) * q, :])
            return t

        xv_tiles[(0, 0)] = load_xv(0, 0, split=4)
        xv_tiles[(0, 1)] = load_xv(0, 1)
        for ch in range(VCH):
            b = ch // (VCH // B)
            vps = [psV.tile([128, 512], F32, name=f"vps{t}", tag=f"v{t}")
                   for t in range(4)]
            for half in range(2):
                # staggered prefetch; chunk 0 delays prefetch to its second
                # half so its own loads don't race the wv pieces
                if ch + 1 < VCH:
                    if half == 0 and ch > 0:
                        xv_tiles[(ch + 1, 0)] = load_xv(ch + 1, 0)
                    if half == 1:
                        if ch == 0:
                            xv_tiles[(1, 0)] = load_xv(1, 0)
                        xv_tiles[(ch + 1, 1)] = load_xv(ch + 1, 1)
                xv = xv_tiles.pop((ch, half))
                for dk16 in range(16):
                    dk = half * 16 + dk16
                    for t in range(4):
                        nc.tensor.matmul(
                            vps[t], xv[:, dk16, t * 128:(t + 1) * 128],
                            wv_sb[:, dk, :],
                            start=(dk == 0), stop=(dk == DKT - 1),
                        )
            if ch == 3:
                # prefetch QK weights while V-pass still runs
                for i in range(4):
                    nc.scalar.dma_start(out=wq_sb[:, i * 8:(i + 1) * 8, :],
                                        in_=wq_d.ap()[:, i * 8:(i + 1) * 8, :])
            if ch == 5:
                for i in range(4):
                    nc.scalar.dma_start(out=wk_sb[:, i * 8:(i + 1) * 8, :],
                                        in_=wk_d.ap()[:, i * 8:(i + 1) * 8, :])
            if ch == 6:
                # stage the first QK sub-chunk's x during the V-pass tail
                nc.scalar.dma_start(out=xqk0_sb, in_=xqk_d.ap()[0, :, :, :])
            for t in range(4):
                vc = vcp.tile([128, 512], F32R, name="vc", tag="vc",
                              bufs=4)
                nc.vector.tensor_copy(vc, vps[t])
                kt = (ch % 4) * 4 + t
                nc.gpsimd.dma_start(out=vdr[b].ap()[kt, :, :], in_=vc)

        for p in (psV, vcp, xvp, wvp):
            p.release()

        # ================= QK-pass =================
        # right-stack order (released top-first): trig + qraw1 + ropet live
        # past qraw0's mid-phase-2 release
        qraw_pool = {}
        qraw_pool[1] = tc.alloc_tile_pool(name="qraw1", bufs=1, side="right")
        trigp = tc.alloc_tile_pool(name="trigp", bufs=1, side="right")
        ropet = tc.alloc_tile_pool(name="ropet", bufs=1, side="right")
        qraw_pool[0] = tc.alloc_tile_pool(name="qraw0", bufs=1, side="right")
        cs_sb = trigp.tile([128, S], F32, name="cs_sb")
        ss_sb = trigp.tile([128, S], F32, name="ss_sb")
        nc.scalar.dma_start(out=cs_sb, in_=cs_d.ap())
        nc.scalar.dma_start(out=ss_sb, in_=ss_d.ap())
        mkd_sb = (qraw_pool[1].tile([128, 4, 512], F32R, name="mkd")
                  if causal else None)
        if causal:
            nc.scalar.dma_start(out=mkd_sb, in_=mk_d.ap())
        xqkp = tc.alloc_tile_pool(name="xqkp", bufs=2, side="left")
        psQK = tc.alloc_tile_pool(name="psQK", bufs=2, space="PSUM")

        qraw = {}
        for b in range(B):
            for qk in ("q", "k"):
                qraw[(qk, b)] = qraw_pool[b].tile(
                    [128, HLOC, S], F16, name=f"raw_{qk}_{b}",
                    tag=f"raw_{qk}_{b}")

        S2 = S // 2
        rope_units = []  # pending (b, hh, qk, col0, width) rope micro-units

        def emit_rope_unit(b, hh, qk, c0, w, eng=None):
            # f32 intermediates (fp16 rope arithmetic costs too much accuracy)
            if eng is None:
                eng = nc.vector
            cols = slice(c0, c0 + w)
            raw = qraw[(qk, b)][:, hh, cols]
            s1 = ropet.tile([128, w], F32, name="s1", tag="s1",
                            padded_shape=[128, S2])
            t1 = ropet.tile([128, w], F32, name="t1", tag="t1",
                            padded_shape=[128, S2])
            s1w = ropet.tile([128, w], F32, name="s1w", tag="s1w",
                             padded_shape=[128, S2])
            eng.tensor_mul(s1, raw, ss_sb[:, cols])
            # partition-half swap; sync queue is idle here
            nc.sync.dma_start(out=s1w[0:64, :], in_=s1[64:128, :])
            nc.sync.dma_start(out=s1w[64:128, :], in_=s1[0:64, :])
            eng.tensor_mul(t1, raw, cs_sb[:, cols])
            eng.tensor_add(raw, t1, s1w)
            if dbg:
                nc.scalar.dma_start(
                    out=qdbg[(qk, b)].ap()[:, hh, cols], in_=raw)

        def pump_rope(n, eng=None):
            for _ in range(min(n, len(rope_units))):
                emit_rope_unit(*rope_units.pop(0), eng=eng)

        xqk_tiles = {}

        def load_xqk(sc):
            t = xqkp.tile([128, DKT, 128], F16, name=f"xqk{sc}", tag="xqk")
            nc.sync.dma_start(out=t, in_=xqk_d.ap()[sc, :, :, :])
            return t

        xqk_tiles[0] = xqk0_sb
        for sc in range(NSC):
            b, col = sc // KT, (sc % KT) * 128
            if sc + 1 < NSC:
                xqk_tiles[sc + 1] = load_xqk(sc + 1)
            xq = xqk_tiles.pop(sc)
            qps = psQK.tile([128, HLOC, 128], F32, name="qps", tag="qps")
            kps = psQK.tile([128, HLOC, 128], F32, name="kps", tag="kps")
            # one accumulation group per PSUM bank: start zeroes the whole
            # 2KB zero region, so only the first matmul into the bank may
            # set start, and only the last sets stop
            for dk in range(DKT):
                for h in range(HLOC):
                    nc.tensor.matmul(
                        qps[:, h, :], wq_sb[:, dk, h * 128:(h + 1) * 128],
                        xq[:, dk, :],
                        start=(dk == 0 and h == 0),
                        stop=(dk == DKT - 1 and h == HLOC - 1))
                for h in range(HLOC):
                    nc.tensor.matmul(
                        kps[:, h, :], wk_sb[:, dk, h * 128:(h + 1) * 128],
                        xq[:, dk, :],
                        start=(dk == 0 and h == 0),
                        stop=(dk == DKT - 1 and h == HLOC - 1))
            nc.vector.tensor_copy(qraw[("q", b)][:, :, col:col + 128], qps)
            nc.scalar.activation(qraw[("k", b)][:, :, col:col + 128],
                                 kps, COPY)
            # rope work drip-fed between sub-chunks so it never head-of-line
            # blocks the DVE drains
            if sc == KT - 1:
                rope_units.extend((0, hh, qk, hf * S2, S2)
                                  for hh in range(HLOC)
                                  for qk in ("q", "k") for hf in (0, 1))
            if sc == KT + KT // 2 - 1:
                rope_units.extend((1, hh, qk, 0, S2) for hh in range(HLOC)
                                  for qk in ("q", "k"))
            if sc == KT + 12 - 1:
                # third quarter of b1's columns is projected by now
                rope_units.extend((1, hh, qk, 2 * 512, 512)
                                  for hh in range(HLOC) for qk in ("q", "k"))
            if KT <= sc < KT + 12:
                # front-loaded: DVE backlog must be clear before phase 2
                pump_rope(2)
            elif sc >= KT + 12:
                pump_rope(2)
        pump_rope(len(rope_units))  # safety flush (empty when on schedule)

        for p in (psQK, xqkp, wqkp):
            p.release()

        # ================= phase 2 + phase 3 =================
        wop = tc.alloc_tile_pool(name="wop", bufs=1, side="left")
        vTp = tc.alloc_tile_pool(name="vTp", bufs=2, side="left")
        sm = tc.alloc_tile_pool(name="sm", bufs=2, side="left")
        psO = tc.alloc_tile_pool(name="psO", bufs=2, space="PSUM")
        psA = tc.alloc_tile_pool(name="psA", bufs=3, space="PSUM")
        psB = tc.alloc_tile_pool(name="psB", bufs=2, space="PSUM")
        psS = tc.alloc_tile_pool(name="psS", bufs=1, space="PSUM")

        wo_sb = wop.tile([128, HLOC, D], F32R, tag="wo")

        def load_wo(i):
            # wo isn't needed until the first p3 group (after unit 4);
            # gpsimd queue keeps the ACT sequencer free for exp dispatch
            nc.gpsimd.dma_start(out=wo_sb[:, i, :], in_=wo_d.ap()[:, i, :])

        hbs = [(b, h) for b in range(B) for h in range(HLOC)]
        vTs = {}

        def load_vT(i):
            b, h = hbs[i]
            t = vTp.tile([128, KT, 128], F32R, name=f"vT{i}", tag="vT")
            nc.sync.dma_start(
                out=t,
                in_=vdr[b].ap()[:, :, h * 128:(h + 1) * 128].rearrange(
                    "k p d -> p k d"))
            return t

        vTs[0] = load_vT(0)

        def p2_unit(i):
            b, h = hbs[i]
            if i + 1 < len(hbs):
                vTs[i + 1] = load_vT(i + 1)
            vT = vTs.pop(i)
            qT = qraw[("q", b)][:, h, :]
            kT = qraw[("k", b)][:, h, :]
            for qc in range(QC):
                qs = qc * 512
                nkt = (qc + 1) * 4 if causal else KT
                # diagonal k-tiles first so their extra DVE mask-mul
                # latency hides behind the off-diagonal tail
                if causal:
                    kts = list(range(nkt - 4, nkt)) + list(range(nkt - 4))
                else:
                    kts = list(range(nkt))
                sums = psS.tile([1, 512], F32, name="sums", tag="sums")
                hops = psB.tile([128, 512], F32, name="hops", tag="hops")
                pend = []
                # pair-summed denominators halve the ones-matmul count; only
                # affordable in the b1 units where the interleaved output
                # projection leaves DVE slack
                use_pairs = i >= HLOC and causal

                def emit_tail(kt, ex, first, last, split):
                    nc.tensor.matmul(hops, vT[:, kt, :], ex,
                                     start=first, stop=last)
                    if use_pairs:
                        p = len(pend_done)
                        pend_done.append(ex)
                        if p % 2 == 1:
                            pr = sm.tile([128, 512], F32R, name="pr",
                                         tag="pr", bufs=1)
                            nc.vector.tensor_add(pr, pend_done[p - 1], ex)
                            nc.tensor.matmul(sums, ones_sb, pr,
                                             start=(p == 1),
                                             stop=(p == len(kts) - 1))
                    else:
                        nc.tensor.matmul(sums, ones_sb, ex,
                                         start=first, stop=last)

                pend_done = []

                for idx, kt in enumerate(kts):
                    split = False
                    st = psA.tile([128, 512], F32, name="st", tag="st")
                    nc.tensor.matmul(st, kT[:, kt * 128:(kt + 1) * 128],
                                     qT[:, qs:qs + 512],
                                     start=True, stop=True)
                    ex = sm.tile([128, 512], F32R, name="ex", tag="ex", bufs=4)
                    diag = causal and kt >= nkt - 4
                    # columns [0, j*128) of a diagonal tile are fully masked:
                    # skip their exp (the mask-mul zeroes them; safe once all
                    # 4 ex buffers hold finite values, i.e. after unit0/qc0)
                    lo = 0
                    if diag and not (i == 0 and qc == 0):
                        lo = (kt - (nkt - 4)) * 128
                    if split:
                        nc.scalar.activation(ex[:, 0:256], st[:, 0:256],
                                             EXP, scale=ISQRT)
                        nc.scalar.activation(ex[:, 256:512], st[:, 256:512],
                                             EXP, scale=ISQRT)
                    elif lo:
                        nc.scalar.activation(ex[:, lo:512], st[:, lo:512],
                                             EXP, scale=ISQRT)
                    else:
                        nc.scalar.activation(ex, st, EXP, scale=ISQRT)
                    if diag or not causal:
                        exm = sm.tile([128, 512], F32R, name="exm", tag="exm",
                                      bufs=4)
                        if causal:
                            j = kt - (nkt - 4)
                            if split:
                                nc.vector.tensor_mul(exm[:, 0:256],
                                                     ex[:, 0:256],
                                                     mkd_sb[:, j, 0:256])
                                nc.vector.tensor_mul(exm[:, 256:512],
                                                     ex[:, 256:512],
                                                     mkd_sb[:, j, 256:512])
                            else:
                                nc.vector.tensor_mul(exm, ex, mkd_sb[:, j, :])
                        else:
                            mkt = sm.tile([128, 512], F32R, name="mkt",
                                          tag="mkt", bufs=2)
                            nc.sync.dma_start(out=mkt,
                                              in_=mk_d.ap()[kt, :, qs:qs + 512])
                            nc.vector.tensor_mul(exm, ex, mkt)
                        ex = exm
                    pend.append((kt, ex, split))
                    if idx >= 2:
                        pkt, pex, psp = pend[idx - 2]
                        emit_tail(pkt, pex, idx - 2 == 0,
                                  idx - 2 == len(kts) - 1, psp)
                for j2 in range(max(0, len(kts) - 2), len(kts)):
                    pkt, pex, psp = pend[j2]
                    emit_tail(pkt, pex, j2 == 0, j2 == len(kts) - 1, psp)

                recip = sm.tile([1, 512], F32, name="recip", tag="recip",
                                bufs=1)
                nc.vector.reciprocal(recip, sums)
                bc = sm.tile([128, 512], F32, name="bc", tag="bc", bufs=1)
                nc.gpsimd.partition_broadcast(bc, recip)
                hstage = sm.tile([128, 512], F32R, name="hstage", tag="hstage",
                                 bufs=2)
                nc.vector.tensor_mul(hstage, hops, bc)
                nc.sync.dma_start(out=hdr[b].ap()[:, h, qs:qs + 512],
                                  in_=hstage)

        osbp = None
        hlp = None
        ops_pool = {"cur": psO, "bufs": 2}
        groups = [(0, i) for i in range(4)] + [(1, i) for i in range(4)]
        hls = {}

        def load_hl(gi):
            b, quarter = groups[gi]
            t0 = quarter * 4
            t = hlp.tile([128, HLOC, 512], F32R, name=f"hl{gi}", tag="hl")
            nc.gpsimd.dma_start(
                out=t, in_=hdr[b].ap()[:, :, t0 * 128:(t0 + 4) * 128])
            return t

        def p3_group(gi):
            b, quarter = groups[gi]
            if gi + 1 < len(groups):
                hls[gi + 1] = load_hl(gi + 1)
            hl = hls.pop(gi)
            for ti in range(4):
                t = quarter * 4 + ti
                osb = osbp.tile([128, D], F32, name="osb", tag="osb")
                for oc in range(D // 512):
                    ops = ops_pool["cur"].tile([128, 512], F32, name="ops",
                                               tag="ops",
                                               bufs=ops_pool["bufs"])
                    for h in range(HLOC):
                        nc.tensor.matmul(
                            ops, hl[:, h, ti * 128:(ti + 1) * 128],
                            wo_sb[:, h, oc * 512:(oc + 1) * 512],
                            start=(h == 0), stop=(h == HLOC - 1))
                    if oc % 2 == 0:
                        nc.vector.tensor_copy(
                            osb[:, oc * 512:(oc + 1) * 512], ops)
                    else:
                        nc.scalar.activation(
                            osb[:, oc * 512:(oc + 1) * 512], ops, COPY)
                row = b * S + t * 128
                if gi == len(groups) - 1 and ti == 3:
                    # shrink the kernel tail: split the last write across
                    # queues so the final transfer+sem chain is short
                    for k4, eng4 in enumerate((nc.sync, nc.scalar,
                                               nc.sync, nc.scalar)):
                        eng4.dma_start(
                            out=out_d.ap()[row:row + 128,
                                           k4 * 1024:(k4 + 1) * 1024],
                            in_=osb[:, k4 * 1024:(k4 + 1) * 1024])
                elif gi == len(groups) - 1 and ti == 2:
                    nc.gpsimd.dma_start(out=out_d.ap()[row:row + 128, 0:2048],
                                        in_=osb[:, 0:2048])
                    nc.sync.dma_start(out=out_d.ap()[row:row + 128, 2048:D],
                                      in_=osb[:, 2048:D])
                else:
                    nc.gpsimd.dma_start(out=out_d.ap()[row:row + 128, :],
                                        in_=osb)

        # batch 0 attention; rope(b1)'s second half spread between units so
        # its DVE work doesn't head-of-line-block phase 2's DVE ops
        for i in range(HLOC):
            p2_unit(i)
            load_wo(i)
            # in-phase-2 rope runs on the (mostly idle) Pool engine so it
            # can't head-of-line-block phase 2's DVE chain
            rope_units.extend((1, i, qk, 3 * 512, 512) for qk in ("q", "k"))
            pump_rope(len(rope_units), eng=nc.gpsimd)
        qraw_pool[0].release()
        ropet.release()
        trigp.release()
        osbp = tc.alloc_tile_pool(name="osbp", bufs=2, side="left")
        hlp = tc.alloc_tile_pool(name="hlp", bufs=2, side="left")
        hls[0] = load_hl(0)
        # batch 1 attention interleaved with batch 0 output projection
        for i in range(HLOC):
            p2_unit(HLOC + i)
            p3_group(i)
        # phase-2 PSUM pools are done; hand their banks to the b1 output
        # projection for deeper pipelining
        for p in (psS, psB, psA):
            p.release()
        psO2 = tc.alloc_tile_pool(name="psO2", bufs=4, space="PSUM")
        ops_pool["cur"], ops_pool["bufs"] = psO2, 4
        for gi in range(4, 8):
            p3_group(gi)

        for p in (psO2, psO, hlp, osbp, sm, vTp, wop,
                  qraw_pool[1], consts):
            p.release()

    nc.compile()
    return nc


def _get_nc(causal: bool):
    if causal not in _CACHE:
        _CACHE[causal] = _build(causal)
    return _CACHE[causal]


def _host_prep(x, wq, wk, wv, wo, freqs_cos, freqs_sin, mask):
    f16 = np.float16
    x2 = np.ascontiguousarray(x.reshape(TOK, D)).astype(f16)

    # xv: [VCH, 128, DKT, 512]; xv[c, p, dk, t] = x[c*512+t, dk*128+p]
    xv = np.ascontiguousarray(
        x2.reshape(VCH, 512, DKT, 128).transpose(0, 3, 2, 1))
    # xqk: [NSC, 128, DKT, 128]; xqk[c, p, dk, t] = x[c*128+t, dk*128+p]
    xqk = np.ascontiguousarray(
        x2.reshape(NSC, 128, DKT, 128).transpose(0, 3, 2, 1))

    cs = np.concatenate([freqs_cos.T, freqs_cos.T], axis=0).astype(np.float32)
    ss = np.concatenate([freqs_sin.T, -freqs_sin.T], axis=0).astype(np.float32)

    m2 = np.asarray(mask, dtype=np.float32).reshape(S, S)
    tril = np.tril(np.ones((S, S), dtype=bool))
    causal = bool(np.all(m2[tril] == 0.0) and np.all(m2[~tril] <= -1e8))
    def expmask(m):
        return np.where(m <= -1e8, 0.0, np.exp(np.maximum(m * ISQRT, -80.0)))

    if causal:
        # exp(mask/sqrt(HD)) on the diagonal band: tile j is the mask for
        # k-tile (nkt-4+j) against a 512-wide q chunk -> 0/1 exactly.
        mk = np.ascontiguousarray(
            expmask(m2[:512, :512].T)
            .reshape(4, 128, 512).transpose(1, 0, 2)).astype(np.float32)
    else:
        mk = np.ascontiguousarray(
            expmask(m2.T).reshape(KT, 128, S)).astype(np.float32)

    perm = np.concatenate([np.arange(0, HD, 2), np.arange(1, HD, 2)])

    in_maps = []
    for c in range(NC):
        cols = np.concatenate([(4 * c + h) * HD + perm for h in range(HLOC)])
        wq_c = np.ascontiguousarray(
            wq[:, cols].reshape(DKT, 128, 512).transpose(1, 0, 2)).astype(f16)
        wk_c = np.ascontiguousarray(
            wk[:, cols].reshape(DKT, 128, 512).transpose(1, 0, 2)).astype(f16)
        vcols = np.arange(4 * c * HD, 4 * (c + 1) * HD)
        wv_c = np.ascontiguousarray(
            wv[:, vcols].reshape(DKT, 128, 512).transpose(1, 0, 2)).astype(f16)
        wo_c = np.ascontiguousarray(
            wo[vcols, :].reshape(HLOC, 128, D).transpose(1, 0, 2)
        ).astype(np.float32)
        m = {"xv": xv, "xqk": xqk, "wq": wq_c, "wk": wk_c, "wv": wv_c,
             "wo": wo_c, "cs": cs, "ss": ss}
        m["maskd" if causal else "maskf"] = mk
        in_maps.append(m)
    return in_maps, causal


def kernel(x, wq, wk, wv, wo, freqs_cos, freqs_sin, mask, **_unused):
    from concourse.bass_utils import run_bass_kernel_spmd

    x = np.asarray(x, dtype=np.float32)
    wq = np.asarray(wq, dtype=np.float32)
    wk = np.asarray(wk, dtype=np.float32)
    wv = np.asarray(wv, dtype=np.float32)
    wo = np.asarray(wo, dtype=np.float32)
    freqs_cos = np.asarray(freqs_cos, dtype=np.float32)
    freqs_sin = np.asarray(freqs_sin, dtype=np.float32)

    in_maps, causal = _host_prep(x, wq, wk, wv, wo, freqs_cos, freqs_sin, mask)
    nc = _get_nc(causal)
    res = run_bass_kernel_spmd(nc, in_maps, list(range(NC)))
    out = res.results[0]["out"]
    for c in range(1, NC):
        out = out + res.results[c]["out"]
    return out.reshape(B, S, D).astype(np.float32)


# revision 4
# speedup vs baseline: 1.0014x; 1.0014x over previous
"""Trainium2 Bass kernel v2: multi-head causal attention with RoPE.

Model (per reference):
  B=2, S=2048, D=4096, H=32 heads, HD=128.
  out = softmax(rope(x@wq) @ rope(x@wk)^T / sqrt(HD) + mask) @ (x@wv) @ wo

Sharding: tensor-parallel over heads. Core c owns heads 4c..4c+3; each core
produces a full-shape partial output, host sums the 8 partials.

v2 design (1407us -> 1126us vs the v1 baseline; rel err 1.55e-2 < 2e-2):
  - Mixed precision tuned against the 2e-2 max-error gate: fp16 for x
    tiles, wq/wk/wv and the SBUF-resident post-rope Q/K (no DRAM spill);
    fp32 PSUM accumulation everywhere; f32 for V, probs (exp can reach
    e^30: fp16/bf16 overflow or round too hard), rope arithmetic, hoT
    (spilled to DRAM - no SBUF room at f32) and wo. bf16 fails the gate
    outright (max err 3.6e-2..9.6e-2 per component). The compiler rejects
    mixed 32/16-bit matmul operands, so pairs upgrade together.
  - Contiguous chunk-major host layouts for x so every load is one large
    DMA: DMA cost is dominated by per-transfer overheads (HWDGE issue
    ~630ns is globally serialized, sem propagation ~900ns); the kernel
    runs ~120 DMAs vs ~1200 in v1.
  - PSUM: start=True zeroes the whole 2KB bank ("zero region"), so banks
    hold one accumulation group at a time; 4 per-head accumulators share
    one bank per q/k per sub-chunk. Ping-pong bufs everywhere; the
    phase-2 pools release mid-stream so the b1 output projection gets 4
    banks.
  - Causal mask applied as a multiply on exp(scores) (exp(s+m) =
    exp(s)*exp(m), exactly 0/1 for the causal mask), keeping the mask off
    the PE->ACT critical path; exp skips fully-masked diagonal columns.
  - Attention inner loop software-pipelined (scores 2 deep ahead of the
    exp-dependent PV/ones matmuls), diagonal tiles first; denominators
    pair-summed on DVE in the b1 units (halves the ones-matmul count
    where the interleaved output projection leaves DVE slack).
  - Engine/queue placement tuned so the ACT sequencer only dispatches
    exps during phase 2 (DMA issues live on sync/gpsimd queues), rope
    runs on DVE during the QK pass but on Pool inside phase 2, and rope
    work is drip-fed between sub-chunks (a burst head-of-line blocks the
    in-order DVE queue and stalls the also-in-order PE).
  - Output projection interleaved with the second batch's attention so
    the PE stays busy while ACT computes exps.
"""

import math
import sys

if "/opt/trn_rl_repo" not in sys.path:
    sys.path.insert(0, "/opt/trn_rl_repo")

import numpy as np

B, S, D, H = 2, 2048, 4096, 32
HD = D // H          # 128
HLOC = 4             # heads per core
NC = 8               # cores
TOK = B * S          # 4096
DKT = D // 128       # 32 contraction tiles
KT = S // 128        # 16 k-tiles per sequence
QC = S // 512        # 4 q-chunks of 512 per sequence
VCH = TOK // 512     # 8 V-pass chunks of 512 tokens
NSC = TOK // 128     # 32 QK-pass sub-chunks of 128 tokens
ISQRT = 1.0 / math.sqrt(HD)

_CACHE = {}


def _build(causal: bool):
    import concourse.bacc as bacc
    import concourse.tile as tile
    from concourse import mybir

    F32 = mybir.dt.float32
    F32R = mybir.dt.float32r
    F16 = mybir.dt.float16
    EXP = mybir.ActivationFunctionType.Exp
    COPY = mybir.ActivationFunctionType.Copy

    nc = bacc.Bacc("TRN2", target_bir_lowering=False, debug=False,
                   num_devices=NC)

    # ---- DRAM inputs ----
    # x tiles partition-major per chunk: one contiguous 8-32KB run per
    # partition per load -> minimal DMA descriptor cost
    xv_d = nc.dram_tensor("xv", [VCH, 128, DKT, 512], F16, kind="ExternalInput")
    xqk_d = nc.dram_tensor("xqk", [NSC, 128, DKT, 128], F16, kind="ExternalInput")
    wq_d = nc.dram_tensor("wq", [128, DKT, 512], F16, kind="ExternalInput")
    wk_d = nc.dram_tensor("wk", [128, DKT, 512], F16, kind="ExternalInput")
    wv_d = nc.dram_tensor("wv", [128, DKT, 512], F16, kind="ExternalInput")
    wo_d = nc.dram_tensor("wo", [128, HLOC, D], F32R, kind="ExternalInput")
    cs_d = nc.dram_tensor("cs", [128, S], F32, kind="ExternalInput")
    ss_d = nc.dram_tensor("ss", [128, S], F32, kind="ExternalInput")
    # exp(mask/sqrt(HD)) factors for the diagonal band (causal) or the
    # full mask (general): multiplied into exp(scores).
    if causal:
        mk_d = nc.dram_tensor("maskd", [128, 4, 512], F32R, kind="ExternalInput")
    else:
        mk_d = nc.dram_tensor("maskf", [KT, 128, S], F32R, kind="ExternalInput")
    out_d = nc.dram_tensor("out", [TOK, D], F32, kind="ExternalOutput")

    import os
    dbg = bool(os.environ.get("K2_DBG"))
    # V spill scratch (f32): [kt, tok%128, dcol]
    vkind = "ExternalOutput" if dbg else "Internal"
    vdr = {b: nc.dram_tensor(f"vdr{b}", [KT, 128, 512], F32R, kind=vkind)
           for b in range(B)}
    # normalized attention output, spilled f32 (no SBUF room at f32)
    hdr = {b: nc.dram_tensor(f"hdbg{b}", [128, HLOC, S], F32R, kind=vkind)
           for b in range(B)}
    qdbg = {}
    if dbg:
        for b in range(B):
            for qk in ("q", "k"):
                qdbg[(qk, b)] = nc.dram_tensor(
                    f"qdbg_{qk}{b}", [128, HLOC, S], F16, kind="ExternalOutput")

    with tile.TileContext(nc) as tc:
        # RIGHT stack: long-lived (consts -> hoT -> qraw b1 -> qraw b0);
        # hoTp/qraw pools are allocated at QK-pass start (a pool reserves
        # its space for its whole lifetime, so allocate late).
        consts = tc.alloc_tile_pool(name="consts", bufs=1, side="right")

        ones_sb = consts.tile([128, 1], F32R)
        nc.vector.memset(ones_sb.bitcast(F32), 1.0)

        # ================= V-pass =================
        # LEFT stack: wqkp (outlives V-pass) under the V-pass pools
        wqkp = tc.alloc_tile_pool(name="wqkp", bufs=1, side="left")
        wvp = tc.alloc_tile_pool(name="wvp", bufs=1, side="left")
        xvp = tc.alloc_tile_pool(name="xvp", bufs=3, side="left")
        vcp = tc.alloc_tile_pool(name="vcp", bufs=2, side="left")
        psV = tc.alloc_tile_pool(name="psV", bufs=2, space="PSUM")

        wv_sb = wvp.tile([128, DKT, 512], F16, tag="wv")
        # wv streamed in pieces, smallest first, so chunk 0 starts ASAP
        _pos = 0
        for w in (2, 2, 4, 4, 4, 4, 4, 4, 4):
            nc.scalar.dma_start(out=wv_sb[:, _pos:_pos + w, :],
                                in_=wv_d.ap()[:, _pos:_pos + w, :])
            _pos += w

        wq_sb = wqkp.tile([128, DKT, 512], F16, tag="wq")
        wk_sb = wqkp.tile([128, DKT, 512], F16, tag="wk")
        xqk0_sb = wqkp.tile([128, DKT, 128], F16, tag="xqk0")

        xv_tiles = {}

        def load_xv(ch, half, split=1):
            t = xvp.tile([128, 16, 512], F16, name=f"xv{ch}_{half}", tag="xv")
            q = 16 // split
            for i in range(split):
                nc.sync.dma_start(
                    out=t[:, i * q:(i + 1) * q, :],
                    in_=xv_d.ap()[ch, :, half * 16 + i * q:
                                  half * 16 + (i + 1) * q, :])
            return t

        xv_tiles[(0, 0)] = load_xv(0, 0, split=4)
        xv_tiles[(0, 1)] = load_xv(0, 1)
        for ch in range(VCH):
            b = ch // (VCH // B)
            vps = [psV.tile([128, 512], F32, name=f"vps{t}", tag=f"v{t}")
                   for t in range(4)]
            for half in range(2):
                # staggered prefetch; chunk 0 delays prefetch to its second
                # half so its own loads don't race the wv pieces
                if ch + 1 < VCH:
                    if half == 0 and ch > 0:
                        xv_tiles[(ch + 1, 0)] = load_xv(ch + 1, 0)
                    if half == 1:
                        if ch == 0:
                            xv_tiles[(1, 0)] = load_xv(1, 0)
                        xv_tiles[(ch + 1, 1)] = load_xv(ch + 1, 1)
                xv = xv_tiles.pop((ch, half))
                for dk16 in range(16):
                    dk = half * 16 + dk16
                    for t in range(4):
                        nc.tensor.matmul(
                            vps[t], xv[:, dk16, t * 128:(t + 1) * 128],
                            wv_sb[:, dk, :],
                            start=(dk == 0), stop=(dk == DKT - 1),
                        )
            # prefetch QK weights while the V-pass runs, one piece per
            # chunk so the transfers don't burst against the x loads
            if 2 <= ch <= 5:
                i = ch - 2
                nc.scalar.dma_start(out=wq_sb[:, i * 8:(i + 1) * 8, :],
                                    in_=wq_d.ap()[:, i * 8:(i + 1) * 8, :])
            if 4 <= ch <= 7:
                i = ch - 4
                nc.scalar.dma_start(out=wk_sb[:, i * 8:(i + 1) * 8, :],
                                    in_=wk_d.ap()[:, i * 8:(i + 1) * 8, :])
            if ch == 7:
                # stage the first QK sub-chunk's x during the V-pass tail
                nc.scalar.dma_start(out=xqk0_sb, in_=xqk_d.ap()[0, :, :, :])
            for t in range(4):
                vc = vcp.tile([128, 512], F32R, name="vc", tag="vc",
                              bufs=4)
                nc.vector.tensor_copy(vc, vps[t])
                kt = (ch % 4) * 4 + t
                nc.gpsimd.dma_start(out=vdr[b].ap()[kt, :, :], in_=vc)

        for p in (psV, vcp, xvp, wvp):
            p.release()

        # ================= QK-pass =================
        # right-stack order (released top-first): trig + qraw1 + ropet live
        # past qraw0's mid-phase-2 release
        qraw_pool = {}
        qraw_pool[1] = tc.alloc_tile_pool(name="qraw1", bufs=1, side="right")
        trigp = tc.alloc_tile_pool(name="trigp", bufs=1, side="right")
        ropet = tc.alloc_tile_pool(name="ropet", bufs=1, side="right")
        qraw_pool[0] = tc.alloc_tile_pool(name="qraw0", bufs=1, side="right")
        cs_sb = trigp.tile([128, S], F32, name="cs_sb")
        ss_sb = trigp.tile([128, S], F32, name="ss_sb")
        nc.scalar.dma_start(out=cs_sb, in_=cs_d.ap())
        nc.scalar.dma_start(out=ss_sb, in_=ss_d.ap())
        mkd_sb = (qraw_pool[1].tile([128, 4, 512], F32R, name="mkd")
                  if causal else None)
        if causal:
            nc.scalar.dma_start(out=mkd_sb, in_=mk_d.ap())
        xqkp = tc.alloc_tile_pool(name="xqkp", bufs=2, side="left")
        psQK = tc.alloc_tile_pool(name="psQK", bufs=2, space="PSUM")

        qraw = {}
        for b in range(B):
            for qk in ("q", "k"):
                qraw[(qk, b)] = qraw_pool[b].tile(
                    [128, HLOC, S], F16, name=f"raw_{qk}_{b}",
                    tag=f"raw_{qk}_{b}")

        S2 = S // 2
        rope_units = []  # pending (b, hh, qk, col0, width) rope micro-units

        def emit_rope_unit(b, hh, qk, c0, w, eng=None):
            # f32 intermediates (fp16 rope arithmetic costs too much accuracy)
            if eng is None:
                eng = nc.vector
            cols = slice(c0, c0 + w)
            raw = qraw[(qk, b)][:, hh, cols]
            s1 = ropet.tile([128, w], F32, name="s1", tag="s1",
                            padded_shape=[128, S2])
            t1 = ropet.tile([128, w], F32, name="t1", tag="t1",
                            padded_shape=[128, S2])
            s1w = ropet.tile([128, w], F32, name="s1w", tag="s1w",
                             padded_shape=[128, S2])
            eng.tensor_mul(s1, raw, ss_sb[:, cols])
            # partition-half swap; sync queue is idle here
            nc.sync.dma_start(out=s1w[0:64, :], in_=s1[64:128, :])
            nc.sync.dma_start(out=s1w[64:128, :], in_=s1[0:64, :])
            eng.tensor_mul(t1, raw, cs_sb[:, cols])
            eng.tensor_add(raw, t1, s1w)
            if dbg:
                nc.scalar.dma_start(
                    out=qdbg[(qk, b)].ap()[:, hh, cols], in_=raw)

        def pump_rope(n, eng=None):
            for _ in range(min(n, len(rope_units))):
                emit_rope_unit(*rope_units.pop(0), eng=eng)

        xqk_tiles = {}

        def load_xqk(sc):
            t = xqkp.tile([128, DKT, 128], F16, name=f"xqk{sc}", tag="xqk")
            nc.sync.dma_start(out=t, in_=xqk_d.ap()[sc, :, :, :])
            return t

        xqk_tiles[0] = xqk0_sb
        for sc in range(NSC):
            b, col = sc // KT, (sc % KT) * 128
            if sc + 1 < NSC:
                xqk_tiles[sc + 1] = load_xqk(sc + 1)
            xq = xqk_tiles.pop(sc)
            qps = psQK.tile([128, HLOC, 128], F32, name="qps", tag="qps")
            kps = psQK.tile([128, HLOC, 128], F32, name="kps", tag="kps")
            # one accumulation group per PSUM bank: start zeroes the whole
            # 2KB zero region, so only the first matmul into the bank may
            # set start, and only the last sets stop
            for dk in range(DKT):
                for h in range(HLOC):
                    nc.tensor.matmul(
                        qps[:, h, :], wq_sb[:, dk, h * 128:(h + 1) * 128],
                        xq[:, dk, :],
                        start=(dk == 0 and h == 0),
                        stop=(dk == DKT - 1 and h == HLOC - 1))
                for h in range(HLOC):
                    nc.tensor.matmul(
                        kps[:, h, :], wk_sb[:, dk, h * 128:(h + 1) * 128],
                        xq[:, dk, :],
                        start=(dk == 0 and h == 0),
                        stop=(dk == DKT - 1 and h == HLOC - 1))
            nc.vector.tensor_copy(qraw[("q", b)][:, :, col:col + 128], qps)
            nc.scalar.activation(qraw[("k", b)][:, :, col:col + 128],
                                 kps, COPY)
            # rope work drip-fed between sub-chunks so it never head-of-line
            # blocks the DVE drains
            if sc == KT - 1:
                rope_units.extend((0, hh, qk, hf * S2, S2)
                                  for hh in range(HLOC)
                                  for qk in ("q", "k") for hf in (0, 1))
            if sc == KT + KT // 2 - 1:
                rope_units.extend((1, hh, qk, 0, S2) for hh in range(HLOC)
                                  for qk in ("q", "k"))
            if sc == KT + 12 - 1:
                # third quarter of b1's columns is projected by now
                rope_units.extend((1, hh, qk, 2 * 512, 512)
                                  for hh in range(HLOC) for qk in ("q", "k"))
            if KT <= sc < KT + 12:
                # front-loaded: DVE backlog must be clear before phase 2
                pump_rope(2)
            elif sc >= KT + 12:
                pump_rope(2)
        pump_rope(len(rope_units))  # safety flush (empty when on schedule)

        for p in (psQK, xqkp, wqkp):
            p.release()

        # ================= phase 2 + phase 3 =================
        wop = tc.alloc_tile_pool(name="wop", bufs=1, side="left")
        vTp = tc.alloc_tile_pool(name="vTp", bufs=2, side="left")
        sm = tc.alloc_tile_pool(name="sm", bufs=2, side="left")
        psO = tc.alloc_tile_pool(name="psO", bufs=2, space="PSUM")
        psA = tc.alloc_tile_pool(name="psA", bufs=3, space="PSUM")
        psB = tc.alloc_tile_pool(name="psB", bufs=2, space="PSUM")
        psS = tc.alloc_tile_pool(name="psS", bufs=1, space="PSUM")

        wo_sb = wop.tile([128, HLOC, D], F32R, tag="wo")

        def load_wo(i):
            # wo isn't needed until the first p3 group (after unit 4);
            # gpsimd queue keeps the ACT sequencer free for exp dispatch
            nc.gpsimd.dma_start(out=wo_sb[:, i, :], in_=wo_d.ap()[:, i, :])

        hbs = [(b, h) for b in range(B) for h in range(HLOC)]
        vTs = {}

        def load_vT(i):
            b, h = hbs[i]
            t = vTp.tile([128, KT, 128], F32R, name=f"vT{i}", tag="vT")
            nc.sync.dma_start(
                out=t,
                in_=vdr[b].ap()[:, :, h * 128:(h + 1) * 128].rearrange(
                    "k p d -> p k d"))
            return t

        vTs[0] = load_vT(0)

        def p2_unit(i):
            b, h = hbs[i]
            if i + 1 < len(hbs):
                vTs[i + 1] = load_vT(i + 1)
            vT = vTs.pop(i)
            qT = qraw[("q", b)][:, h, :]
            kT = qraw[("k", b)][:, h, :]
            for qc in range(QC):
                qs = qc * 512
                nkt = (qc + 1) * 4 if causal else KT
                # diagonal k-tiles first so their extra DVE mask-mul
                # latency hides behind the off-diagonal tail
                if causal:
                    kts = list(range(nkt - 4, nkt)) + list(range(nkt - 4))
                else:
                    kts = list(range(nkt))
                sums = psS.tile([1, 512], F32, name="sums", tag="sums")
                hops = psB.tile([128, 512], F32, name="hops", tag="hops")
                pend = []
                # pair-summed denominators halve the ones-matmul count; only
                # affordable in the b1 units where the interleaved output
                # projection leaves DVE slack
                use_pairs = i >= HLOC and causal

                def emit_tail(kt, ex, first, last, split):
                    nc.tensor.matmul(hops, vT[:, kt, :], ex,
                                     start=first, stop=last)
                    if use_pairs:
                        p = len(pend_done)
                        pend_done.append(ex)
                        if p % 2 == 1:
                            pr = sm.tile([128, 512], F32R, name="pr",
                                         tag="pr", bufs=1)
                            nc.vector.tensor_add(pr, pend_done[p - 1], ex)
                            nc.tensor.matmul(sums, ones_sb, pr,
                                             start=(p == 1),
                                             stop=(p == len(kts) - 1))
                    else:
                        nc.tensor.matmul(sums, ones_sb, ex,
                                         start=first, stop=last)

                pend_done = []

                lag = 2
                for idx, kt in enumerate(kts):
                    split = False
                    st = psA.tile([128, 512], F32, name="st", tag="st")
                    nc.tensor.matmul(st, kT[:, kt * 128:(kt + 1) * 128],
                                     qT[:, qs:qs + 512],
                                     start=True, stop=True)
                    ex = sm.tile([128, 512], F32R, name="ex", tag="ex", bufs=4)
                    diag = causal and kt >= nkt - 4
                    # columns [0, j*128) of a diagonal tile are fully masked:
                    # skip their exp (the mask-mul zeroes them; safe once all
                    # 4 ex buffers hold finite values, i.e. after unit0/qc0)
                    lo = 0
                    if diag and not (i == 0 and qc == 0):
                        lo = (kt - (nkt - 4)) * 128
                    if split:
                        nc.scalar.activation(ex[:, 0:256], st[:, 0:256],
                                             EXP, scale=ISQRT)
                        nc.scalar.activation(ex[:, 256:512], st[:, 256:512],
                                             EXP, scale=ISQRT)
                    elif lo:
                        nc.scalar.activation(ex[:, lo:512], st[:, lo:512],
                                             EXP, scale=ISQRT)
                    else:
                        nc.scalar.activation(ex, st, EXP, scale=ISQRT)
                    if diag or not causal:
                        exm = sm.tile([128, 512], F32R, name="exm", tag="exm",
                                      bufs=4)
                        if causal:
                            j = kt - (nkt - 4)
                            if split:
                                nc.vector.tensor_mul(exm[:, 0:256],
                                                     ex[:, 0:256],
                                                     mkd_sb[:, j, 0:256])
                                nc.vector.tensor_mul(exm[:, 256:512],
                                                     ex[:, 256:512],
                                                     mkd_sb[:, j, 256:512])
                            else:
                                nc.vector.tensor_mul(exm, ex, mkd_sb[:, j, :])
                        else:
                            mkt = sm.tile([128, 512], F32R, name="mkt",
                                          tag="mkt", bufs=2)
                            nc.sync.dma_start(out=mkt,
                                              in_=mk_d.ap()[kt, :, qs:qs + 512])
                            nc.vector.tensor_mul(exm, ex, mkt)
                        ex = exm
                    pend.append((kt, ex, split))
                    if idx >= lag:
                        pkt, pex, psp = pend[idx - lag]
                        emit_tail(pkt, pex, idx - lag == 0,
                                  idx - lag == len(kts) - 1, psp)
                for j2 in range(max(0, len(kts) - lag), len(kts)):
                    pkt, pex, psp = pend[j2]
                    emit_tail(pkt, pex, j2 == 0, j2 == len(kts) - 1, psp)

                recip = sm.tile([1, 512], F32, name="recip", tag="recip",
                                bufs=1)
                nc.vector.reciprocal(recip, sums)
                bc = sm.tile([128, 512], F32, name="bc", tag="bc", bufs=1)
                nc.gpsimd.partition_broadcast(bc, recip)
                hstage = sm.tile([128, 512], F32R, name="hstage", tag="hstage",
                                 bufs=2)
                nc.vector.tensor_mul(hstage, hops, bc)
                nc.sync.dma_start(out=hdr[b].ap()[:, h, qs:qs + 512],
                                  in_=hstage)

        osbp = None
        hlp = None
        ops_pool = {"cur": psO, "bufs": 2}
        groups = [(0, i) for i in range(4)] + [(1, i) for i in range(4)]
        hls = {}

        def load_hl(gi):
            b, quarter = groups[gi]
            t0 = quarter * 4
            t = hlp.tile([128, HLOC, 512], F32R, name=f"hl{gi}", tag="hl")
            nc.gpsimd.dma_start(
                out=t, in_=hdr[b].ap()[:, :, t0 * 128:(t0 + 4) * 128])
            return t

        def p3_group(gi):
            b, quarter = groups[gi]
            for nx in range(gi + 1, min(gi + 3 if gi >= 3 else gi + 2,
                                        len(groups))):
                if nx not in hls:
                    hls[nx] = load_hl(nx)
            hl = hls.pop(gi)
            for ti in range(4):
                t = quarter * 4 + ti
                osb = osbp.tile([128, D], F32, name="osb", tag="osb")
                for oc in range(D // 512):
                    ops = ops_pool["cur"].tile([128, 512], F32, name="ops",
                                               tag="ops",
                                               bufs=ops_pool["bufs"])
                    for h in range(HLOC):
                        nc.tensor.matmul(
                            ops, hl[:, h, ti * 128:(ti + 1) * 128],
                            wo_sb[:, h, oc * 512:(oc + 1) * 512],
                            start=(h == 0), stop=(h == HLOC - 1))
                    if oc % 2 == 0:
                        nc.vector.tensor_copy(
                            osb[:, oc * 512:(oc + 1) * 512], ops)
                    else:
                        nc.scalar.activation(
                            osb[:, oc * 512:(oc + 1) * 512], ops, COPY)
                row = b * S + t * 128
                if gi == len(groups) - 1 and ti == 3:
                    # shrink the kernel tail: split the last write across
                    # queues so the final transfer+sem chain is short
                    for k4, eng4 in enumerate((nc.sync, nc.scalar,
                                               nc.sync, nc.scalar)):
                        eng4.dma_start(
                            out=out_d.ap()[row:row + 128,
                                           k4 * 1024:(k4 + 1) * 1024],
                            in_=osb[:, k4 * 1024:(k4 + 1) * 1024])
                elif gi == len(groups) - 1 and ti == 2:
                    nc.gpsimd.dma_start(out=out_d.ap()[row:row + 128, 0:2048],
                                        in_=osb[:, 0:2048])
                    nc.sync.dma_start(out=out_d.ap()[row:row + 128, 2048:D],
                                      in_=osb[:, 2048:D])
                else:
                    nc.gpsimd.dma_start(out=out_d.ap()[row:row + 128, :],
                                        in_=osb)

        # batch 0 attention; rope(b1)'s second half spread between units so
        # its DVE work doesn't head-of-line-block phase 2's DVE ops
        for i in range(HLOC):
            p2_unit(i)
            load_wo(i)
            # in-phase-2 rope runs on the (mostly idle) Pool engine so it
            # can't head-of-line-block phase 2's DVE chain
            rope_units.extend((1, i, qk, 3 * 512, 512) for qk in ("q", "k"))
            pump_rope(len(rope_units), eng=nc.gpsimd)
        qraw_pool[0].release()
        ropet.release()
        trigp.release()
        osbp = tc.alloc_tile_pool(name="osbp", bufs=2, side="left")
        hlp = tc.alloc_tile_pool(name="hlp", bufs=3, side="left")
        hls[0] = load_hl(0)
        # batch 1 attention interleaved with batch 0 output projection
        for i in range(HLOC):
            p2_unit(HLOC + i)
            p3_group(i)
        # phase-2 PSUM pools are done; hand their banks to the b1 output
        # projection for deeper pipelining
        for p in (psS, psB, psA):
            p.release()
        psO2 = tc.alloc_tile_pool(name="psO2", bufs=4, space="PSUM")
        ops_pool["cur"], ops_pool["bufs"] = psO2, 4
        for gi in range(4, 8):
            p3_group(gi)

        for p in (psO2, psO, hlp, osbp, sm, vTp, wop,
                  qraw_pool[1], consts):
            p.release()

    nc.compile()
    return nc


def _get_nc(causal: bool):
    if causal not in _CACHE:
        _CACHE[causal] = _build(causal)
    return _CACHE[causal]


def _host_prep(x, wq, wk, wv, wo, freqs_cos, freqs_sin, mask):
    f16 = np.float16
    x2 = np.ascontiguousarray(x.reshape(TOK, D)).astype(f16)

    # xv: [VCH, 128, DKT, 512]; xv[c, p, dk, t] = x[c*512+t, dk*128+p]
    xv = np.ascontiguousarray(
        x2.reshape(VCH, 512, DKT, 128).transpose(0, 3, 2, 1))
    # xqk: [NSC, 128, DKT, 128]; xqk[c, p, dk, t] = x[c*128+t, dk*128+p]
    xqk = np.ascontiguousarray(
        x2.reshape(NSC, 128, DKT, 128).transpose(0, 3, 2, 1))

    cs = np.concatenate([freqs_cos.T, freqs_cos.T], axis=0).astype(np.float32)
    ss = np.concatenate([freqs_sin.T, -freqs_sin.T], axis=0).astype(np.float32)

    m2 = np.asarray(mask, dtype=np.float32).reshape(S, S)
    tril = np.tril(np.ones((S, S), dtype=bool))
    causal = bool(np.all(m2[tril] == 0.0) and np.all(m2[~tril] <= -1e8))
    def expmask(m):
        return np.where(m <= -1e8, 0.0, np.exp(np.maximum(m * ISQRT, -80.0)))

    if causal:
        # exp(mask/sqrt(HD)) on the diagonal band: tile j is the mask for
        # k-tile (nkt-4+j) against a 512-wide q chunk -> 0/1 exactly.
        mk = np.ascontiguousarray(
            expmask(m2[:512, :512].T)
            .reshape(4, 128, 512).transpose(1, 0, 2)).astype(np.float32)
    else:
        mk = np.ascontiguousarray(
            expmask(m2.T).reshape(KT, 128, S)).astype(np.float32)

    perm = np.concatenate([np.arange(0, HD, 2), np.arange(1, HD, 2)])

    in_maps = []
    for c in range(NC):
        cols = np.concatenate([(4 * c + h) * HD + perm for h in range(HLOC)])
        wq_c = np.ascontiguousarray(
            wq[:, cols].reshape(DKT, 128, 512).transpose(1, 0, 2)).astype(f16)
        wk_c = np.ascontiguousarray(
            wk[:, cols].reshape(DKT, 128, 512).transpose(1, 0, 2)).astype(f16)
        vcols = np.arange(4 * c * HD, 4 * (c + 1) * HD)
        wv_c = np.ascontiguousarray(
            wv[:, vcols].reshape(DKT, 128, 512).transpose(1, 0, 2)).astype(f16)
        wo_c = np.ascontiguousarray(
            wo[vcols, :].reshape(HLOC, 128, D).transpose(1, 0, 2)
        ).astype(np.float32)
        m = {"xv": xv, "xqk": xqk, "wq": wq_c, "wk": wk_c, "wv": wv_c,
             "wo": wo_c, "cs": cs, "ss": ss}
        m["maskd" if causal else "maskf"] = mk
        in_maps.append(m)
    return in_maps, causal


def kernel(x, wq, wk, wv, wo, freqs_cos, freqs_sin, mask, **_unused):
    from concourse.bass_utils import run_bass_kernel_spmd

    x = np.asarray(x, dtype=np.float32)
    wq = np.asarray(wq, dtype=np.float32)
    wk = np.asarray(wk, dtype=np.float32)
    wv = np.asarray(wv, dtype=np.float32)
    wo = np.asarray(wo, dtype=np.float32)
    freqs_cos = np.asarray(freqs_cos, dtype=np.float32)
    freqs_sin = np.asarray(freqs_sin, dtype=np.float32)

    in_maps, causal = _host_prep(x, wq, wk, wv, wo, freqs_cos, freqs_sin, mask)
    nc = _get_nc(causal)
    res = run_bass_kernel_spmd(nc, in_maps, list(range(NC)))
    out = res.results[0]["out"]
    for c in range(1, NC):
        out = out + res.results[c]["out"]
    return out.reshape(B, S, D).astype(np.float32)


# revision 5
# speedup vs baseline: 1.0134x; 1.0119x over previous
"""Trainium2 Bass kernel v2: multi-head causal attention with RoPE.

Model (per reference):
  B=2, S=2048, D=4096, H=32 heads, HD=128.
  out = softmax(rope(x@wq) @ rope(x@wk)^T / sqrt(HD) + mask) @ (x@wv) @ wo

Sharding: tensor-parallel over heads. Core c owns heads 4c..4c+3; each core
produces a full-shape partial output, host sums the 8 partials.

v2 design (1407us -> 1126us vs the v1 baseline; rel err 1.55e-2 < 2e-2):
  - Mixed precision tuned against the 2e-2 max-error gate: fp16 for x
    tiles, wq/wk/wv and the SBUF-resident post-rope Q/K (no DRAM spill);
    fp32 PSUM accumulation everywhere; f32 for V, probs (exp can reach
    e^30: fp16/bf16 overflow or round too hard), rope arithmetic, hoT
    (spilled to DRAM - no SBUF room at f32) and wo. bf16 fails the gate
    outright (max err 3.6e-2..9.6e-2 per component). The compiler rejects
    mixed 32/16-bit matmul operands, so pairs upgrade together.
  - Contiguous chunk-major host layouts for x so every load is one large
    DMA: DMA cost is dominated by per-transfer overheads (HWDGE issue
    ~630ns is globally serialized, sem propagation ~900ns); the kernel
    runs ~120 DMAs vs ~1200 in v1.
  - PSUM: start=True zeroes the whole 2KB bank ("zero region"), so banks
    hold one accumulation group at a time; 4 per-head accumulators share
    one bank per q/k per sub-chunk. Ping-pong bufs everywhere; the
    phase-2 pools release mid-stream so the b1 output projection gets 4
    banks.
  - Causal mask applied as a multiply on exp(scores) (exp(s+m) =
    exp(s)*exp(m), exactly 0/1 for the causal mask), keeping the mask off
    the PE->ACT critical path; exp skips fully-masked diagonal columns.
  - Attention inner loop software-pipelined (scores 2 deep ahead of the
    exp-dependent PV/ones matmuls), diagonal tiles first; denominators
    pair-summed on DVE in the b1 units (halves the ones-matmul count
    where the interleaved output projection leaves DVE slack).
  - Engine/queue placement tuned so the ACT sequencer only dispatches
    exps during phase 2 (DMA issues live on sync/gpsimd queues), rope
    runs on DVE during the QK pass but on Pool inside phase 2, and rope
    work is drip-fed between sub-chunks (a burst head-of-line blocks the
    in-order DVE queue and stalls the also-in-order PE).
  - Output projection interleaved with the second batch's attention so
    the PE stays busy while ACT computes exps.
"""

import math
import sys

if "/opt/trn_rl_repo" not in sys.path:
    sys.path.insert(0, "/opt/trn_rl_repo")

import numpy as np

B, S, D, H = 2, 2048, 4096, 32
HD = D // H          # 128
HLOC = 4             # heads per core
NC = 8               # cores
TOK = B * S          # 4096
DKT = D // 128       # 32 contraction tiles
KT = S // 128        # 16 k-tiles per sequence
QC = S // 512        # 4 q-chunks of 512 per sequence
VCH = TOK // 512     # 8 V-pass chunks of 512 tokens
NSC = TOK // 128     # 32 QK-pass sub-chunks of 128 tokens
ISQRT = 1.0 / math.sqrt(HD)

_CACHE = {}


def _build(causal: bool):
    import concourse.bacc as bacc
    import concourse.tile as tile
    from concourse import mybir

    F32 = mybir.dt.float32
    F32R = mybir.dt.float32r
    F16 = mybir.dt.float16
    EXP = mybir.ActivationFunctionType.Exp
    COPY = mybir.ActivationFunctionType.Copy

    nc = bacc.Bacc("TRN2", target_bir_lowering=False, debug=False,
                   num_devices=NC)

    # ---- DRAM inputs ----
    # x tiles partition-major per chunk: one contiguous 8-32KB run per
    # partition per load -> minimal DMA descriptor cost
    xv_d = nc.dram_tensor("xv", [VCH, 128, DKT, 512], F16, kind="ExternalInput")
    xqk_d = nc.dram_tensor("xqk", [NSC, 128, DKT, 128], F16, kind="ExternalInput")
    wq_d = nc.dram_tensor("wq", [128, DKT, 512], F16, kind="ExternalInput")
    wk_d = nc.dram_tensor("wk", [128, DKT, 512], F16, kind="ExternalInput")
    wv_d = nc.dram_tensor("wv", [128, DKT, 512], F16, kind="ExternalInput")
    wo_d = nc.dram_tensor("wo", [128, HLOC, D], F32R, kind="ExternalInput")
    cs_d = nc.dram_tensor("cs", [128, S], F32, kind="ExternalInput")
    ss_d = nc.dram_tensor("ss", [128, S], F32, kind="ExternalInput")
    # exp(mask/sqrt(HD)) factors for the diagonal band (causal) or the
    # full mask (general): multiplied into exp(scores).
    if causal:
        mk_d = nc.dram_tensor("maskd", [128, 4, 512], F32R, kind="ExternalInput")
    else:
        mk_d = nc.dram_tensor("maskf", [KT, 128, S], F32R, kind="ExternalInput")
    out_d = nc.dram_tensor("out", [TOK, D], F32, kind="ExternalOutput")

    import os
    dbg = bool(os.environ.get("K2_DBG"))
    # V spill scratch (f32): [kt, tok%128, dcol]
    vkind = "ExternalOutput" if dbg else "Internal"
    vdr = {b: nc.dram_tensor(f"vdr{b}", [KT, 128, 512], F32R, kind=vkind)
           for b in range(B)}
    # normalized attention output, spilled f32 (no SBUF room at f32)
    hdr = {b: nc.dram_tensor(f"hdbg{b}", [128, HLOC, S], F32R, kind=vkind)
           for b in range(B)}
    qdbg = {}
    if dbg:
        for b in range(B):
            for qk in ("q", "k"):
                qdbg[(qk, b)] = nc.dram_tensor(
                    f"qdbg_{qk}{b}", [128, HLOC, S], F16, kind="ExternalOutput")

    with tile.TileContext(nc) as tc:
        # RIGHT stack: long-lived (consts -> hoT -> qraw b1 -> qraw b0);
        # hoTp/qraw pools are allocated at QK-pass start (a pool reserves
        # its space for its whole lifetime, so allocate late).
        consts = tc.alloc_tile_pool(name="consts", bufs=1, side="right")

        ones_sb = consts.tile([128, 1], F32R)
        nc.vector.memset(ones_sb.bitcast(F32), 1.0)

        # ================= V-pass =================
        # LEFT stack: wqkp (outlives V-pass) under the V-pass pools
        wqkp = tc.alloc_tile_pool(name="wqkp", bufs=1, side="left")
        wvp = tc.alloc_tile_pool(name="wvp", bufs=1, side="left")
        xvp = tc.alloc_tile_pool(name="xvp", bufs=3, side="left")
        vcp = tc.alloc_tile_pool(name="vcp", bufs=2, side="left")
        psV = tc.alloc_tile_pool(name="psV", bufs=2, space="PSUM")

        wv_sb = wvp.tile([128, DKT, 512], F16, tag="wv")
        # wv streamed in pieces, smallest first, so chunk 0 starts ASAP
        _pos = 0
        for w in (2, 2, 4, 4, 4, 4, 4, 4, 4):
            nc.scalar.dma_start(out=wv_sb[:, _pos:_pos + w, :],
                                in_=wv_d.ap()[:, _pos:_pos + w, :])
            _pos += w

        wq_sb = wqkp.tile([128, DKT, 512], F16, tag="wq")
        wk_sb = wqkp.tile([128, DKT, 512], F16, tag="wk")
        xqk0_sb = wqkp.tile([128, DKT, 128], F16, tag="xqk0")

        xv_tiles = {}

        def load_xv(ch, half, split=1):
            t = xvp.tile([128, 16, 512], F16, name=f"xv{ch}_{half}", tag="xv")
            q = 16 // split
            for i in range(split):
                nc.sync.dma_start(
                    out=t[:, i * q:(i + 1) * q, :],
                    in_=xv_d.ap()[ch, :, half * 16 + i * q:
                                  half * 16 + (i + 1) * q, :])
            return t

        xv_tiles[(0, 0)] = load_xv(0, 0, split=4)
        xv_tiles[(0, 1)] = load_xv(0, 1)
        for ch in range(VCH):
            b = ch // (VCH // B)
            vps = [psV.tile([128, 512], F32, name=f"vps{t}", tag=f"v{t}")
                   for t in range(4)]
            for half in range(2):
                # staggered prefetch; chunk 0 delays prefetch to its second
                # half so its own loads don't race the wv pieces
                if ch + 1 < VCH:
                    if half == 0 and ch > 0:
                        xv_tiles[(ch + 1, 0)] = load_xv(ch + 1, 0)
                    if half == 1:
                        if ch == 0:
                            xv_tiles[(1, 0)] = load_xv(1, 0)
                        xv_tiles[(ch + 1, 1)] = load_xv(ch + 1, 1)
                xv = xv_tiles.pop((ch, half))
                for dk16 in range(16):
                    dk = half * 16 + dk16
                    for t in range(4):
                        nc.tensor.matmul(
                            vps[t], xv[:, dk16, t * 128:(t + 1) * 128],
                            wv_sb[:, dk, :],
                            start=(dk == 0), stop=(dk == DKT - 1),
                        )
            # prefetch QK weights while the V-pass runs, one piece per
            # chunk so the transfers don't burst against the x loads
            if 2 <= ch <= 5:
                i = ch - 2
                nc.scalar.dma_start(out=wq_sb[:, i * 8:(i + 1) * 8, :],
                                    in_=wq_d.ap()[:, i * 8:(i + 1) * 8, :])
            if 4 <= ch <= 7:
                i = ch - 4
                nc.scalar.dma_start(out=wk_sb[:, i * 8:(i + 1) * 8, :],
                                    in_=wk_d.ap()[:, i * 8:(i + 1) * 8, :])
            if ch == 7:
                # stage the first QK sub-chunk's x during the V-pass tail
                nc.scalar.dma_start(out=xqk0_sb, in_=xqk_d.ap()[0, :, :, :])
            for t in range(4):
                vc = vcp.tile([128, 512], F32R, name="vc", tag="vc",
                              bufs=4)
                nc.vector.tensor_copy(vc, vps[t])
                kt = (ch % 4) * 4 + t
                nc.gpsimd.dma_start(out=vdr[b].ap()[kt, :, :], in_=vc)

        for p in (psV, vcp, xvp, wvp):
            p.release()

        # ================= QK-pass =================
        # right-stack order (released top-first): trig + qraw1 + ropet live
        # past qraw0's mid-phase-2 release
        qraw_pool = {}
        qraw_pool[1] = tc.alloc_tile_pool(name="qraw1", bufs=1, side="right")
        trigp = tc.alloc_tile_pool(name="trigp", bufs=1, side="right")
        ropet = tc.alloc_tile_pool(name="ropet", bufs=1, side="right")
        qraw_pool[0] = tc.alloc_tile_pool(name="qraw0", bufs=1, side="right")
        cs_sb = trigp.tile([128, S], F32, name="cs_sb")
        ss_sb = trigp.tile([128, S], F32, name="ss_sb")
        nc.scalar.dma_start(out=cs_sb, in_=cs_d.ap())
        nc.scalar.dma_start(out=ss_sb, in_=ss_d.ap())
        mkd_sb = (qraw_pool[1].tile([128, 4, 512], F32R, name="mkd")
                  if causal else None)
        if causal:
            nc.scalar.dma_start(out=mkd_sb, in_=mk_d.ap())
        xqkp = tc.alloc_tile_pool(name="xqkp", bufs=2, side="left")
        psQK = tc.alloc_tile_pool(name="psQK", bufs=2, space="PSUM")

        qraw = {}
        for b in range(B):
            for qk in ("q", "k"):
                qraw[(qk, b)] = qraw_pool[b].tile(
                    [128, HLOC, S], F16, name=f"raw_{qk}_{b}",
                    tag=f"raw_{qk}_{b}")

        S2 = S // 2
        rope_units = []  # pending (b, hh, qk, col0, width) rope micro-units

        def emit_rope_unit(b, hh, qk, c0, w, eng=None):
            # f32 intermediates (fp16 rope arithmetic costs too much accuracy)
            if eng is None:
                eng = nc.vector
            cols = slice(c0, c0 + w)
            raw = qraw[(qk, b)][:, hh, cols]
            s1 = ropet.tile([128, w], F32, name="s1", tag="s1",
                            padded_shape=[128, S2])
            t1 = ropet.tile([128, w], F32, name="t1", tag="t1",
                            padded_shape=[128, S2])
            s1w = ropet.tile([128, w], F32, name="s1w", tag="s1w",
                             padded_shape=[128, S2])
            eng.tensor_mul(s1, raw, ss_sb[:, cols])
            # partition-half swap; sync queue is idle here
            nc.sync.dma_start(out=s1w[0:64, :], in_=s1[64:128, :])
            nc.sync.dma_start(out=s1w[64:128, :], in_=s1[0:64, :])
            eng.tensor_mul(t1, raw, cs_sb[:, cols])
            eng.tensor_add(raw, t1, s1w)
            if dbg:
                nc.scalar.dma_start(
                    out=qdbg[(qk, b)].ap()[:, hh, cols], in_=raw)

        def pump_rope(n, eng=None):
            for _ in range(min(n, len(rope_units))):
                emit_rope_unit(*rope_units.pop(0), eng=eng)

        xqk_tiles = {}

        def load_xqk(sc):
            t = xqkp.tile([128, DKT, 128], F16, name=f"xqk{sc}", tag="xqk")
            nc.sync.dma_start(out=t, in_=xqk_d.ap()[sc, :, :, :])
            return t

        xqk_tiles[0] = xqk0_sb
        for sc in range(NSC):
            b, col = sc // KT, (sc % KT) * 128
            if sc + 1 < NSC:
                xqk_tiles[sc + 1] = load_xqk(sc + 1)
            xq = xqk_tiles.pop(sc)
            qps = psQK.tile([128, HLOC, 128], F32, name="qps", tag="qps")
            kps = psQK.tile([128, HLOC, 128], F32, name="kps", tag="kps")
            # one accumulation group per PSUM bank: start zeroes the whole
            # 2KB zero region, so only the first matmul into the bank may
            # set start, and only the last sets stop
            for dk in range(DKT):
                for h in range(HLOC):
                    nc.tensor.matmul(
                        qps[:, h, :], wq_sb[:, dk, h * 128:(h + 1) * 128],
                        xq[:, dk, :],
                        start=(dk == 0 and h == 0),
                        stop=(dk == DKT - 1 and h == HLOC - 1))
                for h in range(HLOC):
                    nc.tensor.matmul(
                        kps[:, h, :], wk_sb[:, dk, h * 128:(h + 1) * 128],
                        xq[:, dk, :],
                        start=(dk == 0 and h == 0),
                        stop=(dk == DKT - 1 and h == HLOC - 1))
            nc.vector.tensor_copy(qraw[("q", b)][:, :, col:col + 128], qps)
            nc.scalar.activation(qraw[("k", b)][:, :, col:col + 128],
                                 kps, COPY)
            # rope work drip-fed between sub-chunks so it never head-of-line
            # blocks the DVE drains
            if sc == KT - 1:
                rope_units.extend((0, hh, qk, hf * S2, S2)
                                  for hh in range(HLOC)
                                  for qk in ("q", "k") for hf in (0, 1))
            if sc == KT + KT // 2 - 1:
                rope_units.extend((1, hh, qk, 0, S2) for hh in range(HLOC)
                                  for qk in ("q", "k"))
            if sc == KT + 12 - 1:
                # third quarter of b1's columns is projected by now
                rope_units.extend((1, hh, qk, 2 * 512, 512)
                                  for hh in range(HLOC) for qk in ("q", "k"))
            if KT <= sc < KT + 12:
                # front-loaded: DVE backlog must be clear before phase 2
                pump_rope(2)
            elif sc >= KT + 12:
                pump_rope(2)
        pump_rope(len(rope_units))  # safety flush (empty when on schedule)

        for p in (psQK, xqkp, wqkp):
            p.release()

        # ================= phase 2 + phase 3 =================
        wop = tc.alloc_tile_pool(name="wop", bufs=1, side="left")
        vTp = tc.alloc_tile_pool(name="vTp", bufs=2, side="left")
        sm = tc.alloc_tile_pool(name="sm", bufs=2, side="left")
        psO = tc.alloc_tile_pool(name="psO", bufs=2, space="PSUM")
        psA = tc.alloc_tile_pool(name="psA", bufs=3, space="PSUM")
        psB = tc.alloc_tile_pool(name="psB", bufs=2, space="PSUM")
        psS = tc.alloc_tile_pool(name="psS", bufs=1, space="PSUM")

        wo_sb = wop.tile([128, HLOC, D], F32R, tag="wo")

        def load_wo(i):
            # wo isn't needed until the first p3 group (after unit 4);
            # gpsimd queue keeps the ACT sequencer free for exp dispatch
            nc.gpsimd.dma_start(out=wo_sb[:, i, :], in_=wo_d.ap()[:, i, :])

        hbs = [(b, h) for b in range(B) for h in range(HLOC)]
        vTs = {}

        def load_vT(i):
            b, h = hbs[i]
            t = vTp.tile([128, KT, 128], F32R, name=f"vT{i}", tag="vT")
            nc.sync.dma_start(
                out=t,
                in_=vdr[b].ap()[:, :, h * 128:(h + 1) * 128].rearrange(
                    "k p d -> p k d"))
            return t

        vTs[0] = load_vT(0)

        def p2_unit(i):
            b, h = hbs[i]
            if i + 1 < len(hbs):
                vTs[i + 1] = load_vT(i + 1)
            vT = vTs.pop(i)
            qT = qraw[("q", b)][:, h, :]
            kT = qraw[("k", b)][:, h, :]
            for qc in range(QC):
                qs = qc * 512
                nkt = (qc + 1) * 4 if causal else KT
                # diagonal k-tiles first so their extra DVE mask-mul
                # latency hides behind the off-diagonal tail
                if causal:
                    kts = list(range(nkt - 4, nkt)) + list(range(nkt - 4))
                else:
                    kts = list(range(nkt))
                sums = psS.tile([1, 512], F32, name="sums", tag="sums")
                hops = psB.tile([128, 512], F32, name="hops", tag="hops")
                pend = []
                # pair-summed denominators halve the ones-matmul count; only
                # affordable in the b1 units where the interleaved output
                # projection leaves DVE slack
                use_pairs = i >= HLOC and causal

                def emit_tail(kt, ex, first, last, split):
                    nc.tensor.matmul(hops, vT[:, kt, :], ex,
                                     start=first, stop=last)
                    if use_pairs:
                        p = len(pend_done)
                        pend_done.append(ex)
                        if p % 2 == 1:
                            pr = sm.tile([128, 512], F32R, name="pr",
                                         tag="pr", bufs=1)
                            nc.vector.tensor_add(pr, pend_done[p - 1], ex)
                            nc.tensor.matmul(sums, ones_sb, pr,
                                             start=(p == 1),
                                             stop=(p == len(kts) - 1))
                    else:
                        nc.tensor.matmul(sums, ones_sb, ex,
                                         start=first, stop=last)

                pend_done = []

                lag = 2
                for idx, kt in enumerate(kts):
                    split = False
                    st = psA.tile([128, 512], F32, name="st", tag="st")
                    nc.tensor.matmul(st, kT[:, kt * 128:(kt + 1) * 128],
                                     qT[:, qs:qs + 512],
                                     start=True, stop=True)
                    ex = sm.tile([128, 512], F32R, name="ex", tag="ex", bufs=4)
                    diag = causal and kt >= nkt - 4
                    # columns [0, j*128) of a diagonal tile are fully masked:
                    # skip their exp (the mask-mul zeroes them; safe once all
                    # 4 ex buffers hold finite values, i.e. after unit0/qc0)
                    lo = 0
                    if diag and not (i == 0 and qc == 0):
                        lo = (kt - (nkt - 4)) * 128
                    if split:
                        nc.scalar.activation(ex[:, 0:256], st[:, 0:256],
                                             EXP, scale=ISQRT)
                        nc.scalar.activation(ex[:, 256:512], st[:, 256:512],
                                             EXP, scale=ISQRT)
                    elif lo:
                        nc.scalar.activation(ex[:, lo:512], st[:, lo:512],
                                             EXP, scale=ISQRT)
                    else:
                        nc.scalar.activation(ex, st, EXP, scale=ISQRT)
                    if diag or not causal:
                        exm = sm.tile([128, 512], F32R, name="exm", tag="exm",
                                      bufs=4)
                        if causal:
                            j = kt - (nkt - 4)
                            nc.vector.tensor_mul(exm, ex, mkd_sb[:, j, :])
                        else:
                            mkt = sm.tile([128, 512], F32R, name="mkt",
                                          tag="mkt", bufs=2)
                            nc.sync.dma_start(out=mkt,
                                              in_=mk_d.ap()[kt, :, qs:qs + 512])
                            nc.vector.tensor_mul(exm, ex, mkt)
                        ex = exm
                    pend.append((kt, ex, split))
                    if idx >= lag:
                        pkt, pex, psp = pend[idx - lag]
                        emit_tail(pkt, pex, idx - lag == 0,
                                  idx - lag == len(kts) - 1, psp)
                for j2 in range(max(0, len(kts) - lag), len(kts)):
                    pkt, pex, psp = pend[j2]
                    emit_tail(pkt, pex, j2 == 0, j2 == len(kts) - 1, psp)

                recip = sm.tile([1, 512], F32, name="recip", tag="recip",
                                bufs=1)
                nc.vector.reciprocal(recip, sums)
                bc = sm.tile([128, 512], F32, name="bc", tag="bc", bufs=1)
                nc.gpsimd.partition_broadcast(bc, recip)
                hstage = sm.tile([128, 512], F32R, name="hstage", tag="hstage",
                                 bufs=2)
                nc.vector.tensor_mul(hstage, hops, bc)
                nc.sync.dma_start(out=hdr[b].ap()[:, h, qs:qs + 512],
                                  in_=hstage)

        osbp = None
        hlp = None
        ops_pool = {"cur": psO, "bufs": 2}
        groups = [(0, i) for i in range(4)] + [(1, i) for i in range(4)]
        hls = {}

        def load_hl(gi):
            b, quarter = groups[gi]
            t0 = quarter * 4
            t = hlp.tile([128, HLOC, 512], F32R, name=f"hl{gi}", tag="hl")
            nc.gpsimd.dma_start(
                out=t, in_=hdr[b].ap()[:, :, t0 * 128:(t0 + 4) * 128])
            return t

        def p3_group(gi):
            b, quarter = groups[gi]
            for nx in range(gi + 1, min(gi + 3 if gi >= 3 else gi + 2,
                                        len(groups))):
                if nx not in hls:
                    hls[nx] = load_hl(nx)
            hl = hls.pop(gi)
            for ti in range(4):
                t = quarter * 4 + ti
                row = b * S + t * 128
                # half-row staging (3 deep): each 2KB half streams out as
                # soon as its four column blocks land, shortening both the
                # group-boundary stalls and the final tail chain
                for hf in range(2):
                    osb = osbp.tile([128, D // 2], F32, name="osb",
                                    tag="osb", bufs=3)
                    for oc4 in range(4):
                        oc = hf * 4 + oc4
                        ops = ops_pool["cur"].tile([128, 512], F32,
                                                   name="ops", tag="ops",
                                                   bufs=ops_pool["bufs"])
                        for h in range(HLOC):
                            nc.tensor.matmul(
                                ops, hl[:, h, ti * 128:(ti + 1) * 128],
                                wo_sb[:, h, oc * 512:(oc + 1) * 512],
                                start=(h == 0), stop=(h == HLOC - 1))
                        if oc % 2 == 0:
                            nc.vector.tensor_copy(
                                osb[:, oc4 * 512:(oc4 + 1) * 512], ops)
                        else:
                            nc.scalar.activation(
                                osb[:, oc4 * 512:(oc4 + 1) * 512], ops, COPY)
                    c0 = hf * (D // 2)
                    if gi == len(groups) - 1 and ti == 3 and hf == 1:
                        nc.sync.dma_start(
                            out=out_d.ap()[row:row + 128, c0:c0 + 1024],
                            in_=osb[:, 0:1024])
                        nc.scalar.dma_start(
                            out=out_d.ap()[row:row + 128, c0 + 1024:D],
                            in_=osb[:, 1024:2048])
                    else:
                        nc.gpsimd.dma_start(
                            out=out_d.ap()[row:row + 128, c0:c0 + D // 2],
                            in_=osb)

        # batch 0 attention; rope(b1)'s second half spread between units so
        # its DVE work doesn't head-of-line-block phase 2's DVE ops
        for i in range(HLOC):
            p2_unit(i)
            load_wo(i)
            # in-phase-2 rope runs on the (mostly idle) Pool engine so it
            # can't head-of-line-block phase 2's DVE chain
            rope_units.extend((1, i, qk, 3 * 512, 512) for qk in ("q", "k"))
            pump_rope(len(rope_units), eng=nc.gpsimd)
        qraw_pool[0].release()
        ropet.release()
        trigp.release()
        osbp = tc.alloc_tile_pool(name="osbp", bufs=2, side="left")
        hlp = tc.alloc_tile_pool(name="hlp", bufs=3, side="left")
        hls[0] = load_hl(0)
        # batch 1 attention interleaved with batch 0 output projection
        for i in range(HLOC):
            p2_unit(HLOC + i)
            p3_group(i)
        # phase-2 PSUM pools are done; hand their banks to the b1 output
        # projection for deeper pipelining
        for p in (psS, psB, psA):
            p.release()
        psO2 = tc.alloc_tile_pool(name="psO2", bufs=4, space="PSUM")
        ops_pool["cur"], ops_pool["bufs"] = psO2, 4
        for gi in range(4, 8):
            p3_group(gi)

        for p in (psO2, psO, hlp, osbp, sm, vTp, wop,
                  qraw_pool[1], consts):
            p.release()

    nc.compile()
    return nc


def _get_nc(causal: bool):
    if causal not in _CACHE:
        _CACHE[causal] = _build(causal)
    return _CACHE[causal]


def _host_prep(x, wq, wk, wv, wo, freqs_cos, freqs_sin, mask):
    f16 = np.float16
    x2 = np.ascontiguousarray(x.reshape(TOK, D)).astype(f16)

    # xv: [VCH, 128, DKT, 512]; xv[c, p, dk, t] = x[c*512+t, dk*128+p]
    xv = np.ascontiguousarray(
        x2.reshape(VCH, 512, DKT, 128).transpose(0, 3, 2, 1))
    # xqk: [NSC, 128, DKT, 128]; xqk[c, p, dk, t] = x[c*128+t, dk*128+p]
    xqk = np.ascontiguousarray(
        x2.reshape(NSC, 128, DKT, 128).transpose(0, 3, 2, 1))

    cs = np.concatenate([freqs_cos.T, freqs_cos.T], axis=0).astype(np.float32)
    ss = np.concatenate([freqs_sin.T, -freqs_sin.T], axis=0).astype(np.float32)

    m2 = np.asarray(mask, dtype=np.float32).reshape(S, S)
    tril = np.tril(np.ones((S, S), dtype=bool))
    causal = bool(np.all(m2[tril] == 0.0) and np.all(m2[~tril] <= -1e8))
    def expmask(m):
        return np.where(m <= -1e8, 0.0, np.exp(np.maximum(m * ISQRT, -80.0)))

    if causal:
        # exp(mask/sqrt(HD)) on the diagonal band: tile j is the mask for
        # k-tile (nkt-4+j) against a 512-wide q chunk -> 0/1 exactly.
        mk = np.ascontiguousarray(
            expmask(m2[:512, :512].T)
            .reshape(4, 128, 512).transpose(1, 0, 2)).astype(np.float32)
    else:
        mk = np.ascontiguousarray(
            expmask(m2.T).reshape(KT, 128, S)).astype(np.float32)

    perm = np.concatenate([np.arange(0, HD, 2), np.arange(1, HD, 2)])

    in_maps = []
    for c in range(NC):
        cols = np.concatenate([(4 * c + h) * HD + perm for h in range(HLOC)])
        wq_c = np.ascontiguousarray(
            wq[:, cols].reshape(DKT, 128, 512).transpose(1, 0, 2)).astype(f16)
        wk_c = np.ascontiguousarray(
            wk[:, cols].reshape(DKT, 128, 512).transpose(1, 0, 2)).astype(f16)
        vcols = np.arange(4 * c * HD, 4 * (c + 1) * HD)
        wv_c = np.ascontiguousarray(
            wv[:, vcols].reshape(DKT, 128, 512).transpose(1, 0, 2)).astype(f16)
        wo_c = np.ascontiguousarray(
            wo[vcols, :].reshape(HLOC, 128, D).transpose(1, 0, 2)
        ).astype(np.float32)
        m = {"xv": xv, "xqk": xqk, "wq": wq_c, "wk": wk_c, "wv": wv_c,
             "wo": wo_c, "cs": cs, "ss": ss}
        m["maskd" if causal else "maskf"] = mk
        in_maps.append(m)
    return in_maps, causal


def kernel(x, wq, wk, wv, wo, freqs_cos, freqs_sin, mask, **_unused):
    from concourse.bass_utils import run_bass_kernel_spmd

    x = np.asarray(x, dtype=np.float32)
    wq = np.asarray(wq, dtype=np.float32)
    wk = np.asarray(wk, dtype=np.float32)
    wv = np.asarray(wv, dtype=np.float32)
    wo = np.asarray(wo, dtype=np.float32)
    freqs_cos = np.asarray(freqs_cos, dtype=np.float32)
    freqs_sin = np.asarray(freqs_sin, dtype=np.float32)

    in_maps, causal = _host_prep(x, wq, wk, wv, wo, freqs_cos, freqs_sin, mask)
    nc = _get_nc(causal)
    res = run_bass_kernel_spmd(nc, in_maps, list(range(NC)))
    out = res.results[0]["out"]
    for c in range(1, NC):
        out = out + res.results[c]["out"]
    return out.reshape(B, S, D).astype(np.float32)
